# revision 4
# baseline (speedup 1.0000x reference)
"""MeshConvPoint Bass/Trainium2 kernel.

Problem (per mesh b of B=8, one NeuronCore each):
    nbr_mean[c,v] = (1/deg[v]) * sum_{d<deg[v]} x[c, nbr_idx[v,d]]
    out[o,v]     = sum_c W[o,c,0]*x[c,v] + W[o,c,1]*nbr_mean[c,v] + b[o]

Device strategy (vertex-major gather via SWDGE dma_gather):
  - x^T stored in DRAM as [NSRC, 64] f32 rows (256B each) with a zero row at
    index V; invalid neighbor slots and pad vertices point at the zero row.
  - Degree-sorted tiling: vertices sorted by degree on the host, grouped into
    128-vertex tiles; tile t has a static slot count s_t = max degree of the
    tile across all 8 cores, so the gather fetches ~mean-degree rows per
    vertex instead of D=12. The program is specialized to the slot profile.
  - Gather order j = (row_off_t + d)*128 + v_local lands a chunk as
    [128 parts = v_local, rows = (tile, slot), 64 ch].
  - Compute per chunk, instruction-count minimized:
      * one batched VectorE reduce + one broadcast multiply per equal-degree
        run of tiles (sum over slots, scaled by 1/deg)
      * per tile: TensorE transpose of the mean to channel-major, ScalarE
        copy into partitions 0..63 of a [128, cw] staging tile whose
        partitions 64..127 were DMA-filled with channel-major x (self term),
        then ONE matmul with Wcat = [W1T; W0T]
      * 4 tiles share one [64, 512] PSUM bank; one ScalarE bias-add per bank
  - Host un-permutes output columns.
"""

import numpy as np

import concourse.bacc as bacc
import concourse.mybir as mybir
from concourse import masks
from concourse.tile import TileContext
from concourse.bass_utils import run_bass_kernel_spmd

B, C, V, D, O = 8, 64, 25000, 12, 64

# per-dma_gather limits: 112*128=14336 indices stays under the ~16K-descriptor
# SWDGE carveout (21504 kills the device); 28 tiles bounds SBUF staging
MAX_CHUNK_ROWS = 112
MAX_CHUNK_TILES = 28


def _plan(v):
    nt = -(-v // 128)  # vertex tiles of 128
    return {
        "V": v,
        "NT": nt,
        "VP": nt * 128,
        "NSRC": ((v + 32) + 31) // 32 * 32,  # zero row at index v
    }


def _chunks_from_slots(slots):
    """Greedily pack tiles into gather chunks (row and tile caps).

    Returns a list of (tile_ids, row_offsets) per chunk. The final chunk is
    tapered into pieces of <= 3 tiles so the pipeline tail (compute after the
    last gather) stays short."""
    chunks = []
    cur, offs, rows = [], [], 0
    for t, s in enumerate(slots):
        if cur and (rows + s > MAX_CHUNK_ROWS or len(cur) >= MAX_CHUNK_TILES):
            chunks.append((cur, offs))
            cur, offs, rows = [], [], 0
        cur.append(t)
        offs.append(rows)
        rows += s
    if cur:
        chunks.append((cur, offs))
    if chunks:
        tail_ids, _ = chunks.pop()
        for i in range(0, len(tail_ids), 3):
            ids = tail_ids[i : i + 3]
            offs, r = [], 0
            for t in ids:
                offs.append(r)
                r += slots[t]
            chunks.append((ids, offs))
    return chunks


def _runs(tile_ids, row_offs, slots):
    """Group chunk-local tiles into runs of equal slot count.

    Yields (i0, n, s, r0): chunk-local start tile, run length, slots, row."""
    i = 0
    while i < len(tile_ids):
        s = slots[tile_ids[i]]
        j = i
        while j < len(tile_ids) and slots[tile_ids[j]] == s:
            j += 1
        yield i, j - i, s, row_offs[i]
        i = j


def build_nc(p, slots):
    f32 = mybir.dt.float32
    chunks = _chunks_from_slots(slots)
    total_idx = 128 * sum(slots)
    idx_cols_total = total_idx // 16

    nc = bacc.Bacc()
    xT = nc.declare_dram_parameter("xT", [p["NSRC"], C], f32, isOutput=False)
    xc_d = nc.declare_dram_parameter("xc", [C, p["VP"]], f32, isOutput=False)
    idx16 = nc.declare_dram_parameter(
        "idx16", [128, idx_cols_total], mybir.dt.int16, isOutput=False
    )
    invdeg = nc.declare_dram_parameter("invdeg", [128, p["NT"]], f32, isOutput=False)
    wcat_d = nc.declare_dram_parameter("wcat", [2 * C, O], f32, isOutput=False)
    bias = nc.declare_dram_parameter("bias", [O, 1], f32, isOutput=False)
    out = nc.declare_dram_parameter("out", [O, p["VP"]], f32, isOutput=True)

    with TileContext(nc) as tc:
        with (
            tc.tile_pool(name="const", bufs=1) as cpool,
            tc.tile_pool(name="idxp", bufs=3) as idxpool,
            tc.tile_pool(name="gp", bufs=2) as gpool,
            tc.tile_pool(name="xcp", bufs=2) as xcpool,
            tc.tile_pool(name="stp", bufs=2) as stpool,
            tc.tile_pool(name="outp", bufs=2) as outpool,
            tc.tile_pool(name="psgp", bufs=4, space="PSUM") as psgpool,
            tc.tile_pool(name="psop", bufs=3, space="PSUM") as psopool,
        ):
            invd = cpool.tile([128, p["NT"]], f32)
            nc.sync.dma_start(out=invd[:, :], in_=invdeg[:, :])
            wcat = cpool.tile([2 * C, O], f32)
            nc.sync.dma_start(out=wcat[:, :], in_=wcat_d[:, :])
            bb = cpool.tile([O, 1], f32)
            nc.sync.dma_start(out=bb[:, :], in_=bias[:, :])
            ident = cpool.tile([128, 128], f32)
            masks.make_identity(nc, ident[:, :])

            idx_off = 0  # running idx column offset into idx16
            for tile_ids, row_offs in chunks:
                ntl = len(tile_ids)
                crows = row_offs[-1] + slots[tile_ids[-1]]
                cidx = crows * 128
                icols = cidx // 16
                cw = ntl * 128
                c0 = tile_ids[0] * 128  # first output column of this chunk

                idxb = idxpool.tile([128, icols], mybir.dt.int16, tag="idxb")
                nc.sync.dma_start(
                    out=idxb[:, :], in_=idx16[:, idx_off : idx_off + icols]
                )
                idx_off += icols
                g = gpool.tile([128, crows, C], f32, tag="g")
                nc.gpsimd.dma_gather(
                    g[:, :, :],
                    xT[:, :],
                    idxb[:, :],
                    cidx,
                    cidx,
                    C,
                    # one packet per instruction deadlocks the SWDGE ring once
                    # descriptors exceed the carveout
                    single_packet=False,
                )
                # staging: partitions 64..127 = channel-major x (self term),
                # partitions 0..63 get the transposed neighbor means per tile
                xcb = xcpool.tile([128, cw], f32, tag="xcb")
                nc.sync.dma_start(out=xcb[64:128, :], in_=xc_d[:, c0 : c0 + cw])
                # batched neighbor mean per equal-degree run
                stb = stpool.tile([128, ntl * C], f32, tag="stb")
                for i0, n, s, r0 in _runs(tile_ids, row_offs, slots):
                    t0 = tile_ids[i0]
                    dst = stb[:, i0 * C : (i0 + n) * C].rearrange(
                        "p (n c) -> p n c", c=C
                    )
                    nc.vector.reduce_sum(
                        out=dst,
                        in_=g[:, r0 : r0 + n * s, :].rearrange(
                            "p (n s) c -> p n c s", s=s
                        ),
                        axis=mybir.AxisListType.X,
                    )
                    nc.vector.tensor_mul(
                        dst,
                        dst,
                        invd[:, t0 : t0 + n].unsqueeze(2).broadcast_to([128, n, C]),
                    )
                outst = outpool.tile([O, cw], f32, tag="outst")
                pso = None
                for i in range(ntl):
                    psg = psgpool.tile([O, 128], f32, tag="psg")
                    nc.tensor.transpose(
                        psg[:, :], stb[:, i * C : (i + 1) * C], ident[:, :]
                    )
                    nc.scalar.copy(xcb[0:64, i * 128 : (i + 1) * 128], psg[:, :])
                    if i % 4 == 0:
                        pso = psopool.tile([O, 512], f32, tag="pso")
                    nc.tensor.matmul(
                        pso[:, (i % 4) * 128 : (i % 4 + 1) * 128],
                        lhsT=wcat[:, :],
                        rhs=xcb[:, i * 128 : (i + 1) * 128],
                        start=True,
                        stop=True,
                    )
                    if i % 4 == 3 or i == ntl - 1:
                        k = i % 4 + 1
                        nc.scalar.add(
                            outst[:, (i - k + 1) * 128 : (i + 1) * 128],
                            pso[:, : k * 128],
                            add=bb[:, 0:1],
                        )
                nc.sync.dma_start(out=out[:, c0 : c0 + cw], in_=outst[:, :])
    nc.finalize()
    return nc


def degree_sort(deg_all, p):
    """Shared tiling across cores: per-core ascending-degree vertex order and
    the per-tile static slot counts (max degree in the tile over all cores)."""
    v, vp, nt = p["V"], p["VP"], p["NT"]
    nb = deg_all.shape[0]
    orders = []
    degs_sorted = []
    for bi in range(nb):
        dfull = np.zeros(vp, np.int64)
        dfull[:v] = deg_all[bi]
        order = np.argsort(dfull, kind="stable")
        orders.append(order)
        degs_sorted.append(dfull[order])
    degs_sorted = np.stack(degs_sorted)  # [nb, vp]
    tile_max = degs_sorted.reshape(nb, nt, 128).max(axis=(0, 2))
    slots = np.maximum(tile_max, 1).astype(int).tolist()
    return orders, slots


def host_prep(x, nbr_idx, deg, W, b, p, orders, slots):
    """Per-core input maps: layout/sharding prep only (no math on x)."""
    v, vp, nt, nsrc = p["V"], p["VP"], p["NT"], p["NSRC"]
    # Wcat rows 0..63 multiply the neighbor mean (partitions 0..63 of the
    # staging tile), rows 64..127 the self features
    wcat = np.concatenate([W[:, :, 1].T, W[:, :, 0].T], axis=0).astype(np.float32)
    bvec = np.ascontiguousarray(b.reshape(O, 1), dtype=np.float32)
    nb = x.shape[0]
    in_maps = []
    for bi in range(nb):
        order = orders[bi]
        valid = order < v
        xT = np.zeros((nsrc, C), np.float32)
        xT[:v] = x[bi].T
        xc = np.zeros((C, vp), np.float32)
        xc[:, valid] = x[bi][:, order[valid]]
        dfull = np.zeros(vp, np.int64)
        dfull[:v] = deg[bi]
        deg_s = dfull[order]  # [vp]
        # neighbor table in sorted order, padded to the static slot profile
        nbr_s = np.full((vp, D), v, np.int32)
        nbr_s[valid] = np.where(
            np.arange(D)[None, :] < deg_s[valid][:, None],
            nbr_idx[bi][order[valid]],
            v,
        )
        # gather index stream: per tile t, slots[t] rows of 128 lanes
        parts = []
        nbr_tiles = nbr_s.reshape(nt, 128, D)
        for t in range(nt):
            parts.append(nbr_tiles[t, :, : slots[t]].T)  # [s_t, 128]
        arr = np.concatenate(parts, axis=0).reshape(-1)
        idx16 = np.tile(
            np.ascontiguousarray(arr.reshape(-1, 16).T).astype(np.int16), (8, 1)
        )
        invdeg = np.ascontiguousarray(
            (1.0 / np.maximum(deg_s, 1).astype(np.float32)).reshape(nt, 128).T
        )
        in_maps.append(
            {
                "xT": xT,
                "xc": xc,
                "idx16": np.ascontiguousarray(idx16),
                "invdeg": invdeg,
                "wcat": wcat,
                "bias": bvec,
            }
        )
    return in_maps


_CACHE = {}
TRACE = False
LAST_RESULT = None
LAST_IN_MAPS = None


def _get_nc(p, slots):
    key = (p["V"], tuple(slots))
    if key not in _CACHE:
        _CACHE[key] = build_nc(p, slots)
    return _CACHE[key]


def kernel(x, nbr_idx, deg, W, b):
    global LAST_RESULT, LAST_IN_MAPS
    x = np.asarray(x, np.float32)
    nbr_idx = np.asarray(nbr_idx, np.int32)
    deg = np.asarray(deg, np.int32)
    W = np.asarray(W, np.float32)
    b = np.asarray(b, np.float32)
    p = _plan(x.shape[2])
    orders, slots = degree_sort(deg, p)
    in_maps = host_prep(x, nbr_idx, deg, W, b, p, orders, slots)
    nc = _get_nc(p, slots)
    try:
        res = run_bass_kernel_spmd(nc, in_maps, list(range(len(in_maps))), trace=TRACE)
    except ModuleNotFoundError:
        res = run_bass_kernel_spmd(nc, in_maps, list(range(len(in_maps))), trace=False)
    LAST_RESULT = res
    LAST_IN_MAPS = in_maps
    v = p["V"]
    outs = []
    for bi, r in enumerate(res.results):
        order = orders[bi]
        valid = order < v
        ob = np.empty((O, v), np.float32)
        ob[:, order[valid]] = r["out"][:, valid]
        outs.append(ob)
    out = np.stack(outs, axis=0)
    return out[..., None].astype(np.float32)



# revision 40
# speedup vs baseline: 1.0857x; 1.0857x over previous
"""MeshConvPoint Bass/Trainium2 kernel.

Problem (per mesh b of B=8, one NeuronCore each):
    nbr_mean[c,v] = (1/deg[v]) * sum_{d<deg[v]} x[c, nbr_idx[v,d]]
    out[o,v]     = sum_c W[o,c,0]*x[c,v] + W[o,c,1]*nbr_mean[c,v] + b[o]

Device strategy (fp8 SWDGE dma_gather at the 7ns/descriptor floor):
  - x^T stored in DRAM as [NSRC, 256] float8_e3m4 rows (64B payload in a
    256B-stride row) with a zero row at index V; invalid slots point at the
    zero row. 64B descriptors ride the DMA min-transfer floor (7ns) instead
    of the 2x small-transfer penalty a 256B f32 row pays (22.8ns) — 3.25x
    less DMA-engine occupancy for the same edge count.
  - Degree-sorted tiling (shared across all 8 cores): vertices sorted by
    degree, 128-vertex tiles, tile t gathers s_t = max tile degree rows per
    vertex. Gather order j = (row_off_t + d)*128 + v_local lands a chunk as
    [128 parts = v_local, rows = (tile, slot), 64 ch] fp8.
  - Per equal-degree run: pairwise-tree adds on DVE (fp8+fp8 -> f16 level 1,
    f16 2x-mode levels after) into f16 sums, then one broadcast multiply by
    1/deg (per-lane, vertex-major).
  - Per 4-tile group: TensorE transposes the f16 means into one f16 PSUM
    bank, one ScalarE copy stages them as matmul rhs, then two accumulating
    matmuls (W1^T @ means + W0^T @ x_cm) and one ScalarE bias-drain.
  - Output stores issue on the Activation queue so they never head-of-line
    block the SP queue that feeds gather index streams.
"""

import numpy as np
import ml_dtypes

import concourse.bacc as bacc
import concourse.mybir as mybir
from concourse import masks
from concourse import ap_utils
from concourse.tile import TileContext
from concourse.bass_utils import run_bass_kernel_spmd

B, C, V, D, O = 8, 64, 25000, 12, 64

# per-dma_gather limits: 112*128=14336 indices stays under the ~16K-descriptor
# SWDGE carveout (21504 kills the device); 16 tiles bounds SBUF staging
MAX_CHUNK_ROWS = 112
MAX_CHUNK_TILES = 16

# runs with slot count >= this use TensorE PSUM accumulation for the slot
# sum; smaller runs use DVE pairwise trees (balances DVE vs PE occupancy)
PE_SUM_MIN_S = 9

XSTEP = 256  # fp8 row stride in elements (256B, the descriptor stride quantum)

f8 = mybir.dt.float8e3
f16 = mybir.dt.float16
np_f8 = ml_dtypes.float8_e3m4


def _plan(v):
    nt = -(-v // 128)  # vertex tiles of 128
    return {
        "V": v,
        "NT": nt,
        "VP": nt * 128,
        "NSRC": ((v + 32) + 31) // 32 * 32,  # zero row at index v
    }


def _chunks_from_slots(slots):
    """Greedily pack tiles into gather chunks (row and tile caps).

    Returns a list of (tile_ids, row_offsets) per chunk. The final chunk is
    tapered into pieces of <= 3 tiles so the pipeline tail (compute after the
    last gather) stays short."""
    chunks = []
    cur, offs, rows = [], [], 0
    for t, s in enumerate(slots):
        if cur and (rows + s > MAX_CHUNK_ROWS or len(cur) >= MAX_CHUNK_TILES):
            chunks.append((cur, offs))
            cur, offs, rows = [], [], 0
        cur.append(t)
        offs.append(rows)
        rows += s
    if cur:
        chunks.append((cur, offs))
    if chunks:
        tail_ids, _ = chunks.pop()
        for i in range(0, len(tail_ids), 3):
            ids = tail_ids[i : i + 3]
            offs, r = [], 0
            for t in ids:
                offs.append(r)
                r += slots[t]
            chunks.append((ids, offs))
    return chunks


def _runs(tile_ids, row_offs, slots):
    """Group chunk-local tiles into runs of equal slot count.

    Yields (i0, n, s, r0): chunk-local start tile, run length, slots, row."""
    i = 0
    while i < len(tile_ids):
        s = slots[tile_ids[i]]
        j = i
        while j < len(tile_ids) and slots[tile_ids[j]] == s:
            j += 1
        yield i, j - i, s, row_offs[i]
        i = j


def dma_gather_raw(
    gp, out_ap, in_ap, idxs_ap, num_idxs, elem_size, elem_step, queue_num=0
):
    """BassGpSimd.dma_gather without the elem_size_bytes%256 assert.

    The descriptor format only quantizes the row STRIDE to 256B units
    (stride_bytes_256); the payload length is free. Verified on hardware for
    64B fp8 and 128B f16 payloads (probe_gather.py)."""
    mb = mybir
    assert idxs_ap.dtype == mb.dt.int16
    assert in_ap.dtype == out_ap.dtype
    dtsz = mb.dt.size(in_ap.dtype)
    assert ap_utils.ap_is_contiguous(out_ap.ap[1:])
    assert ap_utils.ap_is_contiguous(idxs_ap.ap[1:])
    assert in_ap.ap[-1][1] == out_ap.ap[-1][1] == elem_size
    assert out_ap.ap[0][1] * out_ap.ap[1][1] == (num_idxs + 127) // 128 * 128
    assert in_ap.ap[0][0] == elem_step
    stride_bytes = elem_step * dtsz
    assert stride_bytes % 256 == 0
    _in_ap = gp.lower_ap_dma(in_ap, for_custom_bir_dma=True)
    return gp.add_instruction(
        mb.InstDMAGatherAnt(
            name=gp.bass.get_next_instruction_name(),
            ins=[
                *_in_ap,
                gp.lower_ap(idxs_ap),
                gp.lower_val_access(gp.to_reg(num_idxs)),
            ],
            outs=[gp.lower_ap(out_ap)],
            transpose=False,
            num_idxs=num_idxs,
            elem_size=elem_size,
            stride_bytes_256=stride_bytes // 256,
            gen_mode=0,
            single_packet=False,
            queue_num=queue_num,
        )
    )


def _tree_reduce(nc, g, scratch, stb, i0, n, s, r0, scr_off):
    """Sum s slot rows per vertex for a run of n tiles: fp8 g rows ->
    f16 stb[:, i0:(i0+n)*C]. Pairwise adds (level 1 fp8+fp8->f16, then f16);
    odd remainders fold in-place into the last column. Returns scratch cols
    consumed."""
    dst = stb[:, i0 * C : (i0 + n) * C].rearrange("p (n c) -> p n c", c=C)
    if s == 1:
        nc.vector.tensor_copy(dst, g[:, r0 : r0 + n, :])
        return 0
    # rows are slot-major within the run: row r0 + j*n + i = slot j, tile i
    src = g[:, r0 : r0 + n * s, :].rearrange("p (s n) c -> p n s c", n=n)
    m = s
    used = 0
    while m > 1:
        h, odd = m // 2, m % 2
        if h == 1:
            dview = dst.unsqueeze(2)
        else:
            dview = scratch[
                :, (scr_off + used) * C : (scr_off + used + n * h) * C
            ].rearrange("p (n h c) -> p n h c", h=h, c=C)
        nc.vector.tensor_add(
            dview, src[:, :, 0 : 2 * h : 2, :], src[:, :, 1 : 2 * h : 2, :]
        )
        if odd:
            nc.vector.tensor_add(
                dview[:, :, h - 1, :], dview[:, :, h - 1, :], src[:, :, m - 1, :]
            )
        if h > 1:
            used += n * h
        src, m = dview, h
    return used


def build_nc(p, slots):
    f32 = mybir.dt.float32
    chunks = _chunks_from_slots(slots)
    total_idx = 128 * sum(slots)
    idx_cols_total = total_idx // 16

    nc = bacc.Bacc(num_swdge_queues=2)
    xT8 = nc.declare_dram_parameter("xT8", [p["NSRC"], XSTEP], f8, isOutput=False)
    xcm_d = nc.declare_dram_parameter("xcm", [C, p["VP"]], f16, isOutput=False)
    idx16 = nc.declare_dram_parameter(
        "idx16", [128, idx_cols_total], mybir.dt.int16, isOutput=False
    )
    invdeg = nc.declare_dram_parameter("invdeg", [128, p["NT"]], f16, isOutput=False)
    w1t_d = nc.declare_dram_parameter("w1t", [C, O], f16, isOutput=False)
    w0t_d = nc.declare_dram_parameter("w0t", [C, O], f16, isOutput=False)
    bias = nc.declare_dram_parameter("bias", [O, 1], f32, isOutput=False)
    out = nc.declare_dram_parameter("out", [O, p["VP"]], f16, isOutput=True)

    # idx column spans per chunk
    idx_spans = []
    off = 0
    for tile_ids, row_offs in chunks:
        crows = row_offs[-1] + slots[tile_ids[-1]]
        icols = crows * 128 // 16
        idx_spans.append((off, icols))
        off += icols

    with TileContext(nc) as tc:
        with (
            tc.tile_pool(name="const", bufs=1) as cpool,
            tc.tile_pool(name="idxp", bufs=4) as idxpool,
            tc.tile_pool(name="gp", bufs=3) as gpool,
            tc.tile_pool(name="scp", bufs=2) as scpool,
            tc.tile_pool(name="stp", bufs=3) as stpool,
            tc.tile_pool(name="rhp", bufs=3) as rhpool,
            tc.tile_pool(name="outp", bufs=3) as outpool,
            tc.tile_pool(name="pssum", bufs=2, space="PSUM") as pssumpool,
            tc.tile_pool(name="psmean", bufs=2, space="PSUM") as psmeanpool,
            tc.tile_pool(name="psop", bufs=2, space="PSUM") as psopool,
        ):
            idx_tiles = {}

            def issue_idx(ci):
                o, icols = idx_spans[ci]
                t = idxpool.tile([128, icols], mybir.dt.int16, tag="idxb")
                nc.sync.dma_start(out=t[:, :], in_=idx16[:, o : o + icols])
                idx_tiles[ci] = t

            # first two index streams lead the SP queue so gather 0 starts
            # as early as possible; bulk constants follow
            idx_issued = 0
            while idx_issued < min(2, len(chunks)):
                issue_idx(idx_issued)
                idx_issued += 1
            invd = cpool.tile([128, p["NT"]], f16)
            nc.sync.dma_start(out=invd[:, :], in_=invdeg[:, :])
            w1t = cpool.tile([C, O], f16)
            nc.sync.dma_start(out=w1t[:, :], in_=w1t_d[:, :])
            w0t = cpool.tile([C, O], f16)
            nc.sync.dma_start(out=w0t[:, :], in_=w0t_d[:, :])
            bb = cpool.tile([O, 1], f32)
            nc.sync.dma_start(out=bb[:, :], in_=bias[:, :])
            xcm = cpool.tile([C, p["VP"]], f16)
            nc.sync.dma_start(out=xcm[:, :], in_=xcm_d[:, :])
            ident8 = cpool.tile([128, 128], f8)
            masks.make_identity(nc, ident8[:, :])
            ident16 = cpool.tile([128, 128], f16)
            masks.make_identity(nc, ident16[:, :])

            pending = []

            def emit_back(ent):
                rhs, k, c0_, q, outst_, last, ci_, ntl_ = ent
                pso = psopool.tile([O, 512], f32, tag="pso")
                nc.tensor.matmul(
                    pso[:, : k * 128],
                    lhsT=w1t[:, :],
                    rhs=rhs[:, : k * 128],
                    start=True,
                    stop=False,
                )
                nc.tensor.matmul(
                    pso[:, : k * 128],
                    lhsT=w0t[:, :],
                    rhs=xcm[:, c0_ + q * 128 : c0_ + (q + k) * 128],
                    start=False,
                    stop=True,
                )
                nc.scalar.add(
                    outst_[:, q * 128 : (q + k) * 128],
                    pso[:, : k * 128],
                    add=bb[:, 0:1],
                )
                if last:
                    # stores ride the Activation queue mid-kernel (SP must
                    # stay clear for index prefetches); tail chunks use the
                    # then-idle SP queue
                    eng = nc.sync if ci_ >= len(chunks) - 3 else nc.scalar
                    eng.dma_start(
                        out=out[:, c0_ : c0_ + ntl_ * 128], in_=outst_[:, :]
                    )

            for ci, (tile_ids, row_offs) in enumerate(chunks):
                ntl = len(tile_ids)
                crows = row_offs[-1] + slots[tile_ids[-1]]
                cidx = crows * 128
                t0 = tile_ids[0]
                c0 = t0 * 128  # first output column of this chunk

                while idx_issued < len(chunks) and idx_issued <= ci + 3:
                    issue_idx(idx_issued)
                    idx_issued += 1
                idxb = idx_tiles.pop(ci)
                g = gpool.tile([128, crows, C], f8, tag="g")
                dma_gather_raw(
                    nc.gpsimd,
                    g[:, :, :],
                    xT8[:, 0:C],
                    idxb[:, :],
                    cidx,
                    C,
                    XSTEP,
                    queue_num=ci % 2,
                )
                # slot sums: high-degree runs accumulate on TensorE (identity
                # matmuls into PSUM, drained by one DVE mul fused with the
                # 1/deg scale); low-degree runs use DVE pairwise trees
                scratch = scpool.tile([128, (crows + ntl) * C], f16, tag="scr")
                stb = stpool.tile([128, ntl * C], f16, tag="stb")
                runs = list(_runs(tile_ids, row_offs, slots))
                hi = [r for r in runs if r[2] >= PE_SUM_MIN_S]
                lo = [r for r in runs if r[2] < PE_SUM_MIN_S]
                psums = None
                if hi:
                    # indexed by chunk-local tile so accumulation blocks can
                    # split at 8-tile (one PSUM bank) boundaries — a matmul
                    # output must never cross a 2KB bank boundary
                    psums = pssumpool.tile([128, ntl * C], f32, tag="psums")
                    gflat = g[:, :, :].rearrange("p r c -> p (r c)")
                    for i0, n, s, r0 in hi:
                        a = 0
                        while a < n:
                            m = min(n - a, 8 - (i0 + a) % 8)
                            acc = psums[:, (i0 + a) * C : (i0 + a + m) * C]
                            for j in range(s):
                                base = r0 + j * n + a
                                nc.tensor.matmul(
                                    acc,
                                    lhsT=ident8[:, :],
                                    rhs=gflat[:, base * C : (base + m) * C],
                                    start=(j == 0),
                                    stop=(j == s - 1),
                                )
                            a += m
                    for i0, n, s, r0 in hi:
                        nc.vector.tensor_mul(
                            stb[:, i0 * C : (i0 + n) * C].rearrange(
                                "p (n c) -> p n c", c=C
                            ),
                            psums[:, i0 * C : (i0 + n) * C].rearrange(
                                "p (n c) -> p n c", c=C
                            ),
                            invd[:, t0 + i0 : t0 + i0 + n]
                            .unsqueeze(2)
                            .broadcast_to([128, n, C]),
                        )
                scr_off = 0
                for i0, n, s, r0 in lo:
                    scr_off += _tree_reduce(nc, g, scratch, stb, i0, n, s, r0, scr_off)
                for i0, n, s, r0 in lo:
                    stv = stb[:, i0 * C : (i0 + n) * C].rearrange(
                        "p (n c) -> p n c", c=C
                    )
                    nc.vector.tensor_mul(
                        stv,
                        stv,
                        invd[:, t0 + i0 : t0 + i0 + n]
                        .unsqueeze(2)
                        .broadcast_to([128, n, C]),
                    )
                outst = outpool.tile([O, ntl * 128], f16, tag="outst")
                for q in range(0, ntl, 4):
                    k = min(4, ntl - q)
                    psm = psmeanpool.tile([O, 512], f16, tag="psm")
                    for i in range(k):
                        nc.tensor.transpose(
                            psm[:, i * 128 : (i + 1) * 128],
                            stb[:, (q + i) * C : (q + i + 1) * C],
                            ident16[:, :],
                        )
                    rhs = rhpool.tile([O, 512], f16, tag="rhs")
                    nc.scalar.copy(rhs[:, : k * 128], psm[:, : k * 128])
                    pending.append(
                        (rhs, k, c0, q, outst, q + 4 >= ntl, ci, ntl)
                    )
                    # one-group software pipeline: the W-matmuls, bias drain
                    # and store of a group are emitted a group later, so the
                    # PE queue never head-of-line waits on an Activation copy
                    while len(pending) > 1:
                        emit_back(pending.pop(0))
            while pending:
                emit_back(pending.pop(0))
    nc.finalize()
    return nc


def degree_sort(deg_all, p):
    """Shared tiling across cores: per-core ascending-degree vertex order and
    the per-tile static slot counts (max degree in the tile over all cores)."""
    v, vp, nt = p["V"], p["VP"], p["NT"]
    nb = deg_all.shape[0]
    orders = []
    degs_sorted = []
    for bi in range(nb):
        dfull = np.zeros(vp, np.int64)
        dfull[:v] = deg_all[bi]
        order = np.argsort(dfull, kind="stable")
        orders.append(order)
        degs_sorted.append(dfull[order])
    degs_sorted = np.stack(degs_sorted)  # [nb, vp]
    tile_max = degs_sorted.reshape(nb, nt, 128).max(axis=(0, 2))
    slots = np.maximum(tile_max, 1).astype(int).tolist()
    return orders, slots


def host_prep(x, nbr_idx, deg, W, b, p, orders, slots):
    """Per-core input maps: layout/sharding/quantization prep (no math on x)."""
    v, vp, nt, nsrc = p["V"], p["VP"], p["NT"], p["NSRC"]
    w1t = np.ascontiguousarray(W[:, :, 1].T).astype(np.float16)
    w0t = np.ascontiguousarray(W[:, :, 0].T).astype(np.float16)
    bvec = np.ascontiguousarray(b.reshape(O, 1), dtype=np.float32)
    nb = x.shape[0]
    in_maps = []
    for bi in range(nb):
        order = orders[bi]
        valid = order < v
        xT8 = np.zeros((nsrc, XSTEP), np_f8)
        xT8[:v, :C] = x[bi].T.astype(np_f8)
        xcm = np.zeros((C, vp), np.float16)
        xcm[:, valid] = x[bi][:, order[valid]].astype(np.float16)
        dfull = np.zeros(vp, np.int64)
        dfull[:v] = deg[bi]
        deg_s = dfull[order]  # [vp]
        # neighbor table in sorted order, padded to the static slot profile
        nbr_s = np.full((vp, D), v, np.int32)
        nbr_s[valid] = np.where(
            np.arange(D)[None, :] < deg_s[valid][:, None],
            nbr_idx[bi][order[valid]],
            v,
        )
        # gather index stream: slot-major within each equal-degree run so a
        # slot's rows form one contiguous 2D matmul rhs on the device
        parts = []
        nbr_tiles = nbr_s.reshape(nt, 128, D)
        for tile_ids, row_offs in _chunks_from_slots(slots):
            for i0, n, s, r0 in _runs(tile_ids, row_offs, slots):
                tids = tile_ids[i0 : i0 + n]
                blk = nbr_tiles[tids, :, :s]  # [n, 128, s]
                parts.append(blk.transpose(2, 0, 1).reshape(-1, 128))
        arr = np.concatenate(parts, axis=0).reshape(-1)
        idx16 = np.tile(
            np.ascontiguousarray(arr.reshape(-1, 16).T).astype(np.int16), (8, 1)
        )
        invdeg = np.ascontiguousarray(
            (1.0 / np.maximum(deg_s, 1)).reshape(nt, 128).T.astype(np.float16)
        )
        in_maps.append(
            {
                "xT8": xT8,
                "xcm": xcm,
                "idx16": np.ascontiguousarray(idx16),
                "invdeg": invdeg,
                "w1t": w1t,
                "w0t": w0t,
                "bias": bvec,
            }
        )
    return in_maps


_CACHE = {}
TRACE = False
LAST_RESULT = None
LAST_IN_MAPS = None


def _get_nc(p, slots):
    key = (p["V"], tuple(slots))
    if key not in _CACHE:
        _CACHE[key] = build_nc(p, slots)
    return _CACHE[key]


def kernel(x, nbr_idx, deg, W, b):
    global LAST_RESULT, LAST_IN_MAPS
    x = np.asarray(x, np.float32)
    nbr_idx = np.asarray(nbr_idx, np.int32)
    deg = np.asarray(deg, np.int32)
    W = np.asarray(W, np.float32)
    b = np.asarray(b, np.float32)
    p = _plan(x.shape[2])
    orders, slots = degree_sort(deg, p)
    in_maps = host_prep(x, nbr_idx, deg, W, b, p, orders, slots)
    nc = _get_nc(p, slots)
    try:
        res = run_bass_kernel_spmd(nc, in_maps, list(range(len(in_maps))), trace=TRACE)
    except ModuleNotFoundError:
        res = run_bass_kernel_spmd(nc, in_maps, list(range(len(in_maps))), trace=False)
    LAST_RESULT = res
    LAST_IN_MAPS = in_maps
    v = p["V"]
    outs = []
    for bi, r in enumerate(res.results):
        order = orders[bi]
        valid = order < v
        ob = np.empty((O, v), np.float32)
        ob[:, order[valid]] = r["out"][:, valid].astype(np.float32)
        outs.append(ob)
    out = np.stack(outs, axis=0)
    return out[..., None].astype(np.float32)


# revision 45
# speedup vs baseline: 1.1250x; 1.0362x over previous
"""MeshConvPoint Bass/Trainium2 kernel.

Problem (per mesh b of B=8, one NeuronCore each):
    nbr_mean[c,v] = (1/deg[v]) * sum_{d<deg[v]} x[c, nbr_idx[v,d]]
    out[o,v]     = sum_c W[o,c,0]*x[c,v] + W[o,c,1]*nbr_mean[c,v] + b[o]

Device strategy (fp8 SWDGE dma_gather at the 7ns/descriptor floor):
  - x^T stored in DRAM as [NSRC, 256] float8_e3m4 rows (64B payload in a
    256B-stride row) with a zero row at index V; invalid slots point at the
    zero row. 64B descriptors ride the DMA min-transfer floor (7ns) instead
    of the 2x small-transfer penalty a 256B f32 row pays (22.8ns) — 3.25x
    less DMA-engine occupancy for the same edge count.
  - Degree-sorted tiling (shared across all 8 cores): vertices sorted by
    degree, 128-vertex tiles, tile t gathers s_t = max tile degree rows per
    vertex. Gather order j = (row_off_t + d)*128 + v_local lands a chunk as
    [128 parts = v_local, rows = (tile, slot), 64 ch] fp8.
  - Per equal-degree run: pairwise-tree adds on DVE (fp8+fp8 -> f16 level 1,
    f16 2x-mode levels after) into f16 sums, then one broadcast multiply by
    1/deg (per-lane, vertex-major).
  - Per 4-tile group: TensorE transposes the f16 means into one f16 PSUM
    bank, one ScalarE copy stages them as matmul rhs, then two accumulating
    matmuls (W1^T @ means + W0^T @ x_cm) and one ScalarE bias-drain.
  - Output stores issue on the Activation queue so they never head-of-line
    block the SP queue that feeds gather index streams.
"""

import numpy as np
import ml_dtypes

import concourse.bacc as bacc
import concourse.mybir as mybir
from concourse import masks
from concourse import ap_utils
from concourse.tile import TileContext
from concourse.bass_utils import run_bass_kernel_spmd

B, C, V, D, O = 8, 64, 25000, 12, 64

# per-dma_gather limits: 112*128=14336 indices stays under the ~16K-descriptor
# SWDGE carveout (21504 kills the device); 16 tiles bounds SBUF staging
MAX_CHUNK_ROWS = 112
MAX_CHUNK_TILES = 16

# runs with slot count >= this use TensorE PSUM accumulation for the slot
# sum; smaller runs use DVE pairwise trees (balances DVE vs PE occupancy)
PE_SUM_MIN_S = 10

XSTEP = 256  # fp8 row stride in elements (256B, the descriptor stride quantum)

f8 = mybir.dt.float8e3
f16 = mybir.dt.float16
np_f8 = ml_dtypes.float8_e3m4


def _plan(v):
    nt = -(-v // 128)  # vertex tiles of 128
    return {
        "V": v,
        "NT": nt,
        "VP": nt * 128,
        "NSRC": ((v + 32) + 31) // 32 * 32,  # zero row at index v
    }


def _chunks_from_slots(slots):
    """Greedily pack tiles into gather chunks (row and tile caps).

    Returns a list of (tile_ids, row_offsets) per chunk. The final chunk is
    tapered into pieces of <= 3 tiles so the pipeline tail (compute after the
    last gather) stays short."""
    chunks = []
    cur, offs, rows = [], [], 0
    for t, s in enumerate(slots):
        if cur and (rows + s > MAX_CHUNK_ROWS or len(cur) >= MAX_CHUNK_TILES):
            chunks.append((cur, offs))
            cur, offs, rows = [], [], 0
        cur.append(t)
        offs.append(rows)
        rows += s
    if cur:
        chunks.append((cur, offs))
    if chunks:
        tail_ids, _ = chunks.pop()
        for i in range(0, len(tail_ids), 3):
            ids = tail_ids[i : i + 3]
            offs, r = [], 0
            for t in ids:
                offs.append(r)
                r += slots[t]
            chunks.append((ids, offs))
    return chunks


def _runs(tile_ids, row_offs, slots):
    """Group chunk-local tiles into runs of equal slot count.

    Yields (i0, n, s, r0): chunk-local start tile, run length, slots, row."""
    i = 0
    while i < len(tile_ids):
        s = slots[tile_ids[i]]
        j = i
        while j < len(tile_ids) and slots[tile_ids[j]] == s:
            j += 1
        yield i, j - i, s, row_offs[i]
        i = j


def dma_gather_raw(
    gp, out_ap, in_ap, idxs_ap, num_idxs, elem_size, elem_step, queue_num=0
):
    """BassGpSimd.dma_gather without the elem_size_bytes%256 assert.

    The descriptor format only quantizes the row STRIDE to 256B units
    (stride_bytes_256); the payload length is free. Verified on hardware for
    64B fp8 and 128B f16 payloads (probe_gather.py)."""
    mb = mybir
    assert idxs_ap.dtype == mb.dt.int16
    assert in_ap.dtype == out_ap.dtype
    dtsz = mb.dt.size(in_ap.dtype)
    assert ap_utils.ap_is_contiguous(out_ap.ap[1:])
    assert ap_utils.ap_is_contiguous(idxs_ap.ap[1:])
    assert in_ap.ap[-1][1] == out_ap.ap[-1][1] == elem_size
    assert out_ap.ap[0][1] * out_ap.ap[1][1] == (num_idxs + 127) // 128 * 128
    assert in_ap.ap[0][0] == elem_step
    stride_bytes = elem_step * dtsz
    assert stride_bytes % 256 == 0
    _in_ap = gp.lower_ap_dma(in_ap, for_custom_bir_dma=True)
    return gp.add_instruction(
        mb.InstDMAGatherAnt(
            name=gp.bass.get_next_instruction_name(),
            ins=[
                *_in_ap,
                gp.lower_ap(idxs_ap),
                gp.lower_val_access(gp.to_reg(num_idxs)),
            ],
            outs=[gp.lower_ap(out_ap)],
            transpose=False,
            num_idxs=num_idxs,
            elem_size=elem_size,
            stride_bytes_256=stride_bytes // 256,
            gen_mode=0,
            single_packet=False,
            queue_num=queue_num,
        )
    )


def _tree_reduce(nc, g, scratch, stb, i0, n, s, r0, scr_off):
    """Sum s slot rows per vertex for a run of n tiles: fp8 g rows ->
    f16 stb[:, i0:(i0+n)*C]. Pairwise adds (level 1 fp8+fp8->f16, then f16);
    odd remainders fold in-place into the last column. Returns scratch cols
    consumed."""
    dst = stb[:, i0 * C : (i0 + n) * C].rearrange("p (n c) -> p n c", c=C)
    if s == 1:
        nc.vector.tensor_copy(dst, g[:, r0 : r0 + n, :])
        return 0
    # rows are slot-major within the run: row r0 + j*n + i = slot j, tile i
    src = g[:, r0 : r0 + n * s, :].rearrange("p (s n) c -> p n s c", n=n)
    m = s
    used = 0
    while m > 1:
        h, odd = m // 2, m % 2
        if h == 1:
            dview = dst.unsqueeze(2)
        else:
            dview = scratch[
                :, (scr_off + used) * C : (scr_off + used + n * h) * C
            ].rearrange("p (n h c) -> p n h c", h=h, c=C)
        nc.vector.tensor_add(
            dview, src[:, :, 0 : 2 * h : 2, :], src[:, :, 1 : 2 * h : 2, :]
        )
        if odd:
            nc.vector.tensor_add(
                dview[:, :, h - 1, :], dview[:, :, h - 1, :], src[:, :, m - 1, :]
            )
        if h > 1:
            used += n * h
        src, m = dview, h
    return used


def build_nc(p, slots):
    f32 = mybir.dt.float32
    chunks = _chunks_from_slots(slots)
    total_idx = 128 * sum(slots)
    idx_cols_total = total_idx // 16

    nc = bacc.Bacc(num_swdge_queues=2)
    xT8 = nc.declare_dram_parameter("xT8", [p["NSRC"], XSTEP], f8, isOutput=False)
    xcm_d = nc.declare_dram_parameter("xcm", [C, p["VP"]], f16, isOutput=False)
    idx16 = nc.declare_dram_parameter(
        "idx16", [128, idx_cols_total], mybir.dt.int16, isOutput=False
    )
    invdeg = nc.declare_dram_parameter("invdeg", [128, p["NT"]], f16, isOutput=False)
    w1t_d = nc.declare_dram_parameter("w1t", [C, O], f16, isOutput=False)
    w0t_d = nc.declare_dram_parameter("w0t", [C, O], f16, isOutput=False)
    bias = nc.declare_dram_parameter("bias", [O, 1], f32, isOutput=False)
    out = nc.declare_dram_parameter("out", [O, p["VP"]], f16, isOutput=True)

    # idx column spans per chunk
    idx_spans = []
    off = 0
    for tile_ids, row_offs in chunks:
        crows = row_offs[-1] + slots[tile_ids[-1]]
        icols = crows * 128 // 16
        idx_spans.append((off, icols))
        off += icols

    with TileContext(nc) as tc:
        with (
            tc.tile_pool(name="const", bufs=1) as cpool,
            tc.tile_pool(name="idxp", bufs=5) as idxpool,
            tc.tile_pool(name="gp", bufs=3) as gpool,
            tc.tile_pool(name="scp", bufs=2) as scpool,
            tc.tile_pool(name="stp", bufs=3) as stpool,
            tc.tile_pool(name="rhp", bufs=4) as rhpool,
            tc.tile_pool(name="outp", bufs=3) as outpool,
            tc.tile_pool(name="pssum", bufs=2, space="PSUM") as pssumpool,
            tc.tile_pool(name="psmean", bufs=2, space="PSUM") as psmeanpool,
            tc.tile_pool(name="psop", bufs=2, space="PSUM") as psopool,
        ):
            idx_tiles = {}

            def issue_idx(ci):
                o, icols = idx_spans[ci]
                t = idxpool.tile([128, icols], mybir.dt.int16, tag="idxb")
                nc.sync.dma_start(out=t[:, :], in_=idx16[:, o : o + icols])
                idx_tiles[ci] = t

            # first two index streams lead the SP queue so gather 0 starts
            # as early as possible; bulk constants follow
            idx_issued = 0
            while idx_issued < min(2, len(chunks)):
                issue_idx(idx_issued)
                idx_issued += 1
            invd = cpool.tile([128, p["NT"]], f16)
            nc.sync.dma_start(out=invd[:, :], in_=invdeg[:, :])
            w1t = cpool.tile([C, O], f16)
            nc.sync.dma_start(out=w1t[:, :], in_=w1t_d[:, :])
            w0t = cpool.tile([C, O], f16)
            nc.sync.dma_start(out=w0t[:, :], in_=w0t_d[:, :])
            bb = cpool.tile([O, 1], f32)
            nc.sync.dma_start(out=bb[:, :], in_=bias[:, :])
            xcm = cpool.tile([C, p["VP"]], f16)
            nc.sync.dma_start(out=xcm[:, :], in_=xcm_d[:, :])
            ident8 = cpool.tile([128, 128], f8)
            masks.make_identity(nc, ident8[:, :])
            ident16 = cpool.tile([128, 128], f16)
            masks.make_identity(nc, ident16[:, :])

            pending = []

            def emit_back(ent):
                rhs, k, c0_, q, outst_, last, ci_, ntl_ = ent
                pso = psopool.tile([O, 512], f32, tag="pso")
                nc.tensor.matmul(
                    pso[:, : k * 128],
                    lhsT=w1t[:, :],
                    rhs=rhs[:, : k * 128],
                    start=True,
                    stop=False,
                )
                nc.tensor.matmul(
                    pso[:, : k * 128],
                    lhsT=w0t[:, :],
                    rhs=xcm[:, c0_ + q * 128 : c0_ + (q + k) * 128],
                    start=False,
                    stop=True,
                )
                nc.scalar.add(
                    outst_[:, q * 128 : (q + k) * 128],
                    pso[:, : k * 128],
                    add=bb[:, 0:1],
                )
                if last:
                    # stores ride the Activation queue mid-kernel (SP must
                    # stay clear for index prefetches); tail chunks use the
                    # then-idle SP queue
                    eng = nc.sync if ci_ >= len(chunks) - 5 else nc.scalar
                    eng.dma_start(
                        out=out[:, c0_ : c0_ + ntl_ * 128], in_=outst_[:, :]
                    )

            for ci, (tile_ids, row_offs) in enumerate(chunks):
                ntl = len(tile_ids)
                crows = row_offs[-1] + slots[tile_ids[-1]]
                cidx = crows * 128
                t0 = tile_ids[0]
                c0 = t0 * 128  # first output column of this chunk

                while idx_issued < len(chunks) and idx_issued <= ci + 4:
                    issue_idx(idx_issued)
                    idx_issued += 1
                idxb = idx_tiles.pop(ci)
                g = gpool.tile([128, crows, C], f8, tag="g")
                dma_gather_raw(
                    nc.gpsimd,
                    g[:, :, :],
                    xT8[:, 0:C],
                    idxb[:, :],
                    cidx,
                    C,
                    XSTEP,
                    queue_num=ci % 2,
                )
                # slot sums: high-degree runs accumulate on TensorE (identity
                # matmuls into PSUM, drained by one DVE mul fused with the
                # 1/deg scale); low-degree runs use DVE pairwise trees
                scratch = scpool.tile([128, (crows + ntl) * C], f16, tag="scr")
                stb = stpool.tile([128, ntl * C], f16, tag="stb")
                runs = list(_runs(tile_ids, row_offs, slots))
                hi = [r for r in runs if r[2] >= PE_SUM_MIN_S]
                lo = [r for r in runs if r[2] < PE_SUM_MIN_S]
                psums = None
                if hi:
                    # indexed by chunk-local tile so accumulation blocks can
                    # split at 8-tile (one PSUM bank) boundaries — a matmul
                    # output must never cross a 2KB bank boundary
                    psums = pssumpool.tile([128, ntl * C], f32, tag="psums")
                    gflat = g[:, :, :].rearrange("p r c -> p (r c)")
                    for i0, n, s, r0 in hi:
                        a = 0
                        while a < n:
                            m = min(n - a, 8 - (i0 + a) % 8)
                            acc = psums[:, (i0 + a) * C : (i0 + a + m) * C]
                            for j in range(s):
                                base = r0 + j * n + a
                                nc.tensor.matmul(
                                    acc,
                                    lhsT=ident8[:, :],
                                    rhs=gflat[:, base * C : (base + m) * C],
                                    start=(j == 0),
                                    stop=(j == s - 1),
                                )
                            a += m
                    for i0, n, s, r0 in hi:
                        nc.vector.tensor_mul(
                            stb[:, i0 * C : (i0 + n) * C].rearrange(
                                "p (n c) -> p n c", c=C
                            ),
                            psums[:, i0 * C : (i0 + n) * C].rearrange(
                                "p (n c) -> p n c", c=C
                            ),
                            invd[:, t0 + i0 : t0 + i0 + n]
                            .unsqueeze(2)
                            .broadcast_to([128, n, C]),
                        )
                scr_off = 0
                for i0, n, s, r0 in lo:
                    scr_off += _tree_reduce(nc, g, scratch, stb, i0, n, s, r0, scr_off)
                for i0, n, s, r0 in lo:
                    stv = stb[:, i0 * C : (i0 + n) * C].rearrange(
                        "p (n c) -> p n c", c=C
                    )
                    nc.vector.tensor_mul(
                        stv,
                        stv,
                        invd[:, t0 + i0 : t0 + i0 + n]
                        .unsqueeze(2)
                        .broadcast_to([128, n, C]),
                    )
                outst = outpool.tile([O, ntl * 128], f16, tag="outst")
                for q in range(0, ntl, 4):
                    k = min(4, ntl - q)
                    psm = psmeanpool.tile([O, 512], f16, tag="psm")
                    for i in range(k):
                        nc.tensor.transpose(
                            psm[:, i * 128 : (i + 1) * 128],
                            stb[:, (q + i) * C : (q + i + 1) * C],
                            ident16[:, :],
                        )
                    rhs = rhpool.tile([O, 512], f16, tag="rhs")
                    nc.scalar.copy(rhs[:, : k * 128], psm[:, : k * 128])
                    pending.append(
                        (rhs, k, c0, q, outst, q + 4 >= ntl, ci, ntl)
                    )
                    # one-group software pipeline: the W-matmuls, bias drain
                    # and store of a group are emitted a group later, so the
                    # PE queue never head-of-line waits on an Activation copy
                    while len(pending) > 1:
                        emit_back(pending.pop(0))
            while pending:
                emit_back(pending.pop(0))
    nc.finalize()
    return nc


def degree_sort(deg_all, p):
    """Shared tiling across cores: per-core ascending-degree vertex order and
    the per-tile static slot counts (max degree in the tile over all cores)."""
    v, vp, nt = p["V"], p["VP"], p["NT"]
    nb = deg_all.shape[0]
    orders = []
    degs_sorted = []
    for bi in range(nb):
        dfull = np.zeros(vp, np.int64)
        dfull[:v] = deg_all[bi]
        order = np.argsort(dfull, kind="stable")
        orders.append(order)
        degs_sorted.append(dfull[order])
    degs_sorted = np.stack(degs_sorted)  # [nb, vp]
    tile_max = degs_sorted.reshape(nb, nt, 128).max(axis=(0, 2))
    slots = np.maximum(tile_max, 1).astype(int).tolist()
    return orders, slots


def host_prep(x, nbr_idx, deg, W, b, p, orders, slots):
    """Per-core input maps: layout/sharding/quantization prep (no math on x)."""
    v, vp, nt, nsrc = p["V"], p["VP"], p["NT"], p["NSRC"]
    w1t = np.ascontiguousarray(W[:, :, 1].T).astype(np.float16)
    w0t = np.ascontiguousarray(W[:, :, 0].T).astype(np.float16)
    bvec = np.ascontiguousarray(b.reshape(O, 1), dtype=np.float32)
    nb = x.shape[0]
    in_maps = []
    for bi in range(nb):
        order = orders[bi]
        valid = order < v
        xT8 = np.zeros((nsrc, XSTEP), np_f8)
        xT8[:v, :C] = x[bi].T.astype(np_f8)
        xcm = np.zeros((C, vp), np.float16)
        xcm[:, valid] = x[bi][:, order[valid]].astype(np.float16)
        dfull = np.zeros(vp, np.int64)
        dfull[:v] = deg[bi]
        deg_s = dfull[order]  # [vp]
        # neighbor table in sorted order, padded to the static slot profile
        nbr_s = np.full((vp, D), v, np.int32)
        nbr_s[valid] = np.where(
            np.arange(D)[None, :] < deg_s[valid][:, None],
            nbr_idx[bi][order[valid]],
            v,
        )
        # gather index stream: slot-major within each equal-degree run so a
        # slot's rows form one contiguous 2D matmul rhs on the device
        parts = []
        nbr_tiles = nbr_s.reshape(nt, 128, D)
        for tile_ids, row_offs in _chunks_from_slots(slots):
            for i0, n, s, r0 in _runs(tile_ids, row_offs, slots):
                tids = tile_ids[i0 : i0 + n]
                blk = nbr_tiles[tids, :, :s]  # [n, 128, s]
                parts.append(blk.transpose(2, 0, 1).reshape(-1, 128))
        arr = np.concatenate(parts, axis=0).reshape(-1)
        idx16 = np.tile(
            np.ascontiguousarray(arr.reshape(-1, 16).T).astype(np.int16), (8, 1)
        )
        invdeg = np.ascontiguousarray(
            (1.0 / np.maximum(deg_s, 1)).reshape(nt, 128).T.astype(np.float16)
        )
        in_maps.append(
            {
                "xT8": xT8,
                "xcm": xcm,
                "idx16": np.ascontiguousarray(idx16),
                "invdeg": invdeg,
                "w1t": w1t,
                "w0t": w0t,
                "bias": bvec,
            }
        )
    return in_maps


_CACHE = {}
TRACE = False
LAST_RESULT = None
LAST_IN_MAPS = None


def _get_nc(p, slots):
    key = (p["V"], tuple(slots))
    if key not in _CACHE:
        _CACHE[key] = build_nc(p, slots)
    return _CACHE[key]


def kernel(x, nbr_idx, deg, W, b):
    global LAST_RESULT, LAST_IN_MAPS
    x = np.asarray(x, np.float32)
    nbr_idx = np.asarray(nbr_idx, np.int32)
    deg = np.asarray(deg, np.int32)
    W = np.asarray(W, np.float32)
    b = np.asarray(b, np.float32)
    p = _plan(x.shape[2])
    orders, slots = degree_sort(deg, p)
    in_maps = host_prep(x, nbr_idx, deg, W, b, p, orders, slots)
    nc = _get_nc(p, slots)
    try:
        res = run_bass_kernel_spmd(nc, in_maps, list(range(len(in_maps))), trace=TRACE)
    except ModuleNotFoundError:
        res = run_bass_kernel_spmd(nc, in_maps, list(range(len(in_maps))), trace=False)
    LAST_RESULT = res
    LAST_IN_MAPS = in_maps
    v = p["V"]
    outs = []
    for bi, r in enumerate(res.results):
        order = orders[bi]
        valid = order < v
        ob = np.empty((O, v), np.float32)
        ob[:, order[valid]] = r["out"][:, valid].astype(np.float32)
        outs.append(ob)
    out = np.stack(outs, axis=0)
    return out[..., None].astype(np.float32)


# revision 51
# speedup vs baseline: 1.1477x; 1.0202x over previous
"""MeshConvPoint Bass/Trainium2 kernel.

Problem (per mesh b of B=8, one NeuronCore each):
    nbr_mean[c,v] = (1/deg[v]) * sum_{d<deg[v]} x[c, nbr_idx[v,d]]
    out[o,v]     = sum_c W[o,c,0]*x[c,v] + W[o,c,1]*nbr_mean[c,v] + b[o]

Device strategy (fp8 SWDGE dma_gather at the 7ns/descriptor floor):
  - x^T stored in DRAM as [NSRC, 256] float8_e3m4 rows (64B payload in a
    256B-stride row) with a zero row at index V; invalid slots point at the
    zero row. 64B descriptors ride the DMA min-transfer floor (7ns) instead
    of the 2x small-transfer penalty a 256B f32 row pays (22.8ns) — 3.25x
    less DMA-engine occupancy for the same edge count.
  - Degree-sorted tiling (shared across all 8 cores): vertices sorted by
    degree, 128-vertex tiles, tile t gathers s_t = max tile degree rows per
    vertex. Gather order j = (row_off_t + d)*128 + v_local lands a chunk as
    [128 parts = v_local, rows = (tile, slot), 64 ch] fp8.
  - Per equal-degree run: pairwise-tree adds on DVE (fp8+fp8 -> f16 level 1,
    f16 2x-mode levels after) into f16 sums, then one broadcast multiply by
    1/deg (per-lane, vertex-major).
  - Per 4-tile group: TensorE transposes the f16 means into one f16 PSUM
    bank, one ScalarE copy stages them as matmul rhs, then two accumulating
    matmuls (W1^T @ means + W0^T @ x_cm) and one ScalarE bias-drain.
  - Output stores issue on the Activation queue so they never head-of-line
    block the SP queue that feeds gather index streams.
"""

import numpy as np
import ml_dtypes

import concourse.bacc as bacc
import concourse.mybir as mybir
from concourse import masks
from concourse import ap_utils
from concourse.tile import TileContext
from concourse.bass_utils import run_bass_kernel_spmd

B, C, V, D, O = 8, 64, 25000, 12, 64

# per-dma_gather limits: 112*128=14336 indices stays under the ~16K-descriptor
# SWDGE carveout (21504 kills the device); 16 tiles bounds SBUF staging
MAX_CHUNK_ROWS = 112
MAX_CHUNK_TILES = 16

# runs with slot count >= this use TensorE PSUM accumulation for the slot
# sum; smaller runs use DVE pairwise trees (balances DVE vs PE occupancy)
PE_SUM_MIN_S = 10

XSTEP = 256  # fp8 row stride in elements (256B, the descriptor stride quantum)

f8 = mybir.dt.float8e3
f16 = mybir.dt.float16
np_f8 = ml_dtypes.float8_e3m4


def _plan(v):
    nt = -(-v // 128)  # vertex tiles of 128
    return {
        "V": v,
        "NT": nt,
        "VP": nt * 128,
        "NSRC": ((v + 32) + 31) // 32 * 32,  # zero row at index v
    }


def _chunks_from_slots(slots):
    """Greedily pack tiles into gather chunks (row and tile caps).

    Returns a list of (tile_ids, row_offsets) per chunk. The final chunk is
    tapered into pieces of <= 3 tiles so the pipeline tail (compute after the
    last gather) stays short."""
    chunks = []
    cur, offs, rows = [], [], 0
    for t, s in enumerate(slots):
        if cur and (rows + s > MAX_CHUNK_ROWS or len(cur) >= MAX_CHUNK_TILES):
            chunks.append((cur, offs))
            cur, offs, rows = [], [], 0
        cur.append(t)
        offs.append(rows)
        rows += s
    if cur:
        chunks.append((cur, offs))
    if chunks:
        tail_ids, _ = chunks.pop()
        for i in range(0, len(tail_ids), 3):
            ids = tail_ids[i : i + 3]
            offs, r = [], 0
            for t in ids:
                offs.append(r)
                r += slots[t]
            chunks.append((ids, offs))
    return chunks


def _runs(tile_ids, row_offs, slots):
    """Group chunk-local tiles into runs of equal slot count.

    Yields (i0, n, s, r0): chunk-local start tile, run length, slots, row."""
    i = 0
    while i < len(tile_ids):
        s = slots[tile_ids[i]]
        j = i
        while j < len(tile_ids) and slots[tile_ids[j]] == s:
            j += 1
        yield i, j - i, s, row_offs[i]
        i = j


def dma_gather_raw(
    gp, out_ap, in_ap, idxs_ap, num_idxs, elem_size, elem_step, queue_num=0
):
    """BassGpSimd.dma_gather without the elem_size_bytes%256 assert.

    The descriptor format only quantizes the row STRIDE to 256B units
    (stride_bytes_256); the payload length is free. Verified on hardware for
    64B fp8 and 128B f16 payloads (probe_gather.py)."""
    mb = mybir
    assert idxs_ap.dtype == mb.dt.int16
    assert in_ap.dtype == out_ap.dtype
    dtsz = mb.dt.size(in_ap.dtype)
    assert ap_utils.ap_is_contiguous(out_ap.ap[1:])
    assert ap_utils.ap_is_contiguous(idxs_ap.ap[1:])
    assert in_ap.ap[-1][1] == out_ap.ap[-1][1] == elem_size
    assert out_ap.ap[0][1] * out_ap.ap[1][1] == (num_idxs + 127) // 128 * 128
    assert in_ap.ap[0][0] == elem_step
    stride_bytes = elem_step * dtsz
    assert stride_bytes % 256 == 0
    _in_ap = gp.lower_ap_dma(in_ap, for_custom_bir_dma=True)
    return gp.add_instruction(
        mb.InstDMAGatherAnt(
            name=gp.bass.get_next_instruction_name(),
            ins=[
                *_in_ap,
                gp.lower_ap(idxs_ap),
                gp.lower_val_access(gp.to_reg(num_idxs)),
            ],
            outs=[gp.lower_ap(out_ap)],
            transpose=False,
            num_idxs=num_idxs,
            elem_size=elem_size,
            stride_bytes_256=stride_bytes // 256,
            gen_mode=0,
            single_packet=False,
            queue_num=queue_num,
        )
    )


def _tree_reduce(nc, g, scratch, stb, i0, n, s, r0, scr_off):
    """Sum s slot rows per vertex for a run of n tiles: fp8 g rows ->
    f16 stb[:, i0:(i0+n)*C]. Pairwise adds (level 1 fp8+fp8->f16, then f16);
    odd remainders fold in-place into the last column. Returns scratch cols
    consumed."""
    dst = stb[:, i0 * C : (i0 + n) * C].rearrange("p (n c) -> p n c", c=C)
    if s == 1:
        nc.vector.tensor_copy(dst, g[:, r0 : r0 + n, :])
        return 0
    # rows are slot-major within the run: row r0 + j*n + i = slot j, tile i
    src = g[:, r0 : r0 + n * s, :].rearrange("p (s n) c -> p n s c", n=n)
    m = s
    used = 0
    while m > 1:
        h, odd = m // 2, m % 2
        if h == 1:
            dview = dst.unsqueeze(2)
        else:
            dview = scratch[
                :, (scr_off + used) * C : (scr_off + used + n * h) * C
            ].rearrange("p (n h c) -> p n h c", h=h, c=C)
        nc.vector.tensor_add(
            dview, src[:, :, 0 : 2 * h : 2, :], src[:, :, 1 : 2 * h : 2, :]
        )
        if odd:
            nc.vector.tensor_add(
                dview[:, :, h - 1, :], dview[:, :, h - 1, :], src[:, :, m - 1, :]
            )
        if h > 1:
            used += n * h
        src, m = dview, h
    return used


def build_nc(p, slots):
    f32 = mybir.dt.float32
    chunks = _chunks_from_slots(slots)
    total_idx = 128 * sum(slots)
    idx_cols_total = total_idx // 16

    nc = bacc.Bacc(num_swdge_queues=2)
    xT8 = nc.declare_dram_parameter("xT8", [p["NSRC"], XSTEP], f8, isOutput=False)
    xcm_d = nc.declare_dram_parameter("xcm", [C, p["VP"]], f16, isOutput=False)
    idx16 = nc.declare_dram_parameter(
        "idx16", [128, idx_cols_total], mybir.dt.int16, isOutput=False
    )
    # consts packed in one tensor: [invdeg NT | w1t 64 | w0t 64 | bias 1]
    cw = p["NT"] + 2 * O + 1
    consts_d = nc.declare_dram_parameter("consts", [128, cw], f16, isOutput=False)
    out = nc.declare_dram_parameter("out", [O, p["VP"]], f16, isOutput=True)

    # idx column spans per chunk
    idx_spans = []
    off = 0
    for tile_ids, row_offs in chunks:
        crows = row_offs[-1] + slots[tile_ids[-1]]
        icols = crows * 128 // 16
        idx_spans.append((off, icols))
        off += icols

    with TileContext(nc) as tc:
        with (
            tc.tile_pool(name="const", bufs=1) as cpool,
            tc.tile_pool(name="idxp", bufs=5) as idxpool,
            tc.tile_pool(name="gp", bufs=3) as gpool,
            tc.tile_pool(name="scp", bufs=2) as scpool,
            tc.tile_pool(name="stp", bufs=3) as stpool,
            tc.tile_pool(name="rhp", bufs=4) as rhpool,
            tc.tile_pool(name="outp", bufs=3) as outpool,
            tc.tile_pool(name="pssum", bufs=2, space="PSUM") as pssumpool,
            tc.tile_pool(name="psmean", bufs=2, space="PSUM") as psmeanpool,
            tc.tile_pool(name="psop", bufs=2, space="PSUM") as psopool,
        ):
            idx_tiles = {}

            def issue_idx(ci):
                o, icols = idx_spans[ci]
                t = idxpool.tile([128, icols], mybir.dt.int16, tag="idxb")
                nc.sync.dma_start(out=t[:, :], in_=idx16[:, o : o + icols])
                idx_tiles[ci] = t

            # first two index streams lead the SP queue so gather 0 starts
            # as early as possible; bulk constants follow
            idx_issued = 0
            while idx_issued < min(2, len(chunks)):
                issue_idx(idx_issued)
                idx_issued += 1
            cb = cpool.tile([128, cw], f16)
            nc.sync.dma_start(out=cb[:, :], in_=consts_d[:, :])
            nt = p["NT"]
            xcm = cpool.tile([C, p["VP"]], f16)
            nc.sync.dma_start(out=xcm[:, :], in_=xcm_d[:, :])
            ident8 = cpool.tile([128, 128], f8)
            masks.make_identity(nc, ident8[:, :])
            ident16 = cpool.tile([128, 128], f16)
            masks.make_identity(nc, ident16[:, :])

            pending = []

            def emit_back(ent):
                rhs, k, c0_, q, outst_, last, ci_, ntl_ = ent
                pso = psopool.tile([O, 512], f32, tag="pso")
                nc.tensor.matmul(
                    pso[:, : k * 128],
                    lhsT=cb[0:C, nt : nt + O],
                    rhs=rhs[:, : k * 128],
                    start=True,
                    stop=False,
                )
                nc.tensor.matmul(
                    pso[:, : k * 128],
                    lhsT=cb[0:C, nt + O : nt + 2 * O],
                    rhs=xcm[:, c0_ + q * 128 : c0_ + (q + k) * 128],
                    start=False,
                    stop=True,
                )
                nc.scalar.add(
                    outst_[:, q * 128 : (q + k) * 128],
                    pso[:, : k * 128],
                    add=cb[0:O, nt + 2 * O : nt + 2 * O + 1],
                )
                if last:
                    # stores ride the Activation queue mid-kernel (SP must
                    # stay clear for index prefetches); tail chunks use the
                    # then-idle SP queue
                    eng = nc.sync if ci_ >= len(chunks) - 5 else nc.scalar
                    eng.dma_start(
                        out=out[:, c0_ : c0_ + ntl_ * 128], in_=outst_[:, :]
                    )

            for ci, (tile_ids, row_offs) in enumerate(chunks):
                ntl = len(tile_ids)
                crows = row_offs[-1] + slots[tile_ids[-1]]
                cidx = crows * 128
                t0 = tile_ids[0]
                c0 = t0 * 128  # first output column of this chunk

                while idx_issued < len(chunks) and idx_issued <= ci + 4:
                    issue_idx(idx_issued)
                    idx_issued += 1
                idxb = idx_tiles.pop(ci)
                g = gpool.tile([128, crows, C], f8, tag="g")
                dma_gather_raw(
                    nc.gpsimd,
                    g[:, :, :],
                    xT8[:, 0:C],
                    idxb[:, :],
                    cidx,
                    C,
                    XSTEP,
                    queue_num=ci % 2,
                )
                # slot sums: high-degree runs accumulate on TensorE (identity
                # matmuls into PSUM, drained by one DVE mul fused with the
                # 1/deg scale); low-degree runs use DVE pairwise trees
                scratch = scpool.tile([128, (crows + ntl) * C], f16, tag="scr")
                stb = stpool.tile([128, ntl * C], f16, tag="stb")
                runs = list(_runs(tile_ids, row_offs, slots))
                hi = [r for r in runs if r[2] >= PE_SUM_MIN_S]
                lo = [r for r in runs if r[2] < PE_SUM_MIN_S]
                psums = None
                if hi:
                    # indexed by chunk-local tile so accumulation blocks can
                    # split at 8-tile (one PSUM bank) boundaries — a matmul
                    # output must never cross a 2KB bank boundary
                    psums = pssumpool.tile([128, ntl * C], f32, tag="psums")
                    gflat = g[:, :, :].rearrange("p r c -> p (r c)")
                    for i0, n, s, r0 in hi:
                        a = 0
                        while a < n:
                            m = min(n - a, 8 - (i0 + a) % 8)
                            acc = psums[:, (i0 + a) * C : (i0 + a + m) * C]
                            for j in range(s):
                                base = r0 + j * n + a
                                nc.tensor.matmul(
                                    acc,
                                    lhsT=ident8[:, :],
                                    rhs=gflat[:, base * C : (base + m) * C],
                                    start=(j == 0),
                                    stop=(j == s - 1),
                                )
                            a += m
                    for i0, n, s, r0 in hi:
                        nc.vector.tensor_mul(
                            stb[:, i0 * C : (i0 + n) * C].rearrange(
                                "p (n c) -> p n c", c=C
                            ),
                            psums[:, i0 * C : (i0 + n) * C].rearrange(
                                "p (n c) -> p n c", c=C
                            ),
                            cb[:, t0 + i0 : t0 + i0 + n]
                            .unsqueeze(2)
                            .broadcast_to([128, n, C]),
                        )
                scr_off = 0
                for i0, n, s, r0 in lo:
                    scr_off += _tree_reduce(nc, g, scratch, stb, i0, n, s, r0, scr_off)
                for i0, n, s, r0 in lo:
                    stv = stb[:, i0 * C : (i0 + n) * C].rearrange(
                        "p (n c) -> p n c", c=C
                    )
                    nc.vector.tensor_mul(
                        stv,
                        stv,
                        cb[:, t0 + i0 : t0 + i0 + n]
                        .unsqueeze(2)
                        .broadcast_to([128, n, C]),
                    )
                outst = outpool.tile([O, ntl * 128], f16, tag="outst")
                for q in range(0, ntl, 4):
                    k = min(4, ntl - q)
                    psm = psmeanpool.tile([O, 512], f16, tag="psm")
                    for i in range(k):
                        nc.tensor.transpose(
                            psm[:, i * 128 : (i + 1) * 128],
                            stb[:, (q + i) * C : (q + i + 1) * C],
                            ident16[:, :],
                        )
                    rhs = rhpool.tile([O, 512], f16, tag="rhs")
                    nc.scalar.copy(rhs[:, : k * 128], psm[:, : k * 128])
                    pending.append(
                        (rhs, k, c0, q, outst, q + 4 >= ntl, ci, ntl)
                    )
                    # one-group software pipeline: the W-matmuls, bias drain
                    # and store of a group are emitted a group later, so the
                    # PE queue never head-of-line waits on an Activation copy
                    while len(pending) > 1:
                        emit_back(pending.pop(0))
            while pending:
                emit_back(pending.pop(0))
    nc.finalize()
    return nc


def degree_sort(deg_all, p):
    """Shared tiling across cores: per-core ascending-degree vertex order and
    the per-tile static slot counts (max degree in the tile over all cores)."""
    v, vp, nt = p["V"], p["VP"], p["NT"]
    nb = deg_all.shape[0]
    orders = []
    degs_sorted = []
    for bi in range(nb):
        dfull = np.zeros(vp, np.int64)
        dfull[:v] = deg_all[bi]
        order = np.argsort(dfull, kind="stable")
        orders.append(order)
        degs_sorted.append(dfull[order])
    degs_sorted = np.stack(degs_sorted)  # [nb, vp]
    tile_max = degs_sorted.reshape(nb, nt, 128).max(axis=(0, 2))
    slots = np.maximum(tile_max, 1).astype(int).tolist()
    return orders, slots


def host_prep(x, nbr_idx, deg, W, b, p, orders, slots):
    """Per-core input maps: layout/sharding/quantization prep (no math on x)."""
    v, vp, nt, nsrc = p["V"], p["VP"], p["NT"], p["NSRC"]
    w1t = np.ascontiguousarray(W[:, :, 1].T).astype(np.float16)
    w0t = np.ascontiguousarray(W[:, :, 0].T).astype(np.float16)
    nb = x.shape[0]
    in_maps = []
    for bi in range(nb):
        order = orders[bi]
        valid = order < v
        xT8 = np.zeros((nsrc, XSTEP), np_f8)
        xT8[:v, :C] = x[bi].T.astype(np_f8)
        xcm = np.zeros((C, vp), np.float16)
        xcm[:, valid] = x[bi][:, order[valid]].astype(np.float16)
        dfull = np.zeros(vp, np.int64)
        dfull[:v] = deg[bi]
        deg_s = dfull[order]  # [vp]
        # neighbor table in sorted order, padded to the static slot profile
        nbr_s = np.full((vp, D), v, np.int32)
        nbr_s[valid] = np.where(
            np.arange(D)[None, :] < deg_s[valid][:, None],
            nbr_idx[bi][order[valid]],
            v,
        )
        # gather index stream: slot-major within each equal-degree run so a
        # slot's rows form one contiguous 2D matmul rhs on the device
        parts = []
        nbr_tiles = nbr_s.reshape(nt, 128, D)
        for tile_ids, row_offs in _chunks_from_slots(slots):
            for i0, n, s, r0 in _runs(tile_ids, row_offs, slots):
                tids = tile_ids[i0 : i0 + n]
                blk = nbr_tiles[tids, :, :s]  # [n, 128, s]
                parts.append(blk.transpose(2, 0, 1).reshape(-1, 128))
        arr = np.concatenate(parts, axis=0).reshape(-1)
        idx16 = np.tile(
            np.ascontiguousarray(arr.reshape(-1, 16).T).astype(np.int16), (8, 1)
        )
        consts = np.zeros((128, nt + 2 * O + 1), np.float16)
        consts[:, :nt] = (1.0 / np.maximum(deg_s, 1)).reshape(nt, 128).T
        consts[:C, nt : nt + O] = w1t
        consts[:C, nt + O : nt + 2 * O] = w0t
        consts[:O, nt + 2 * O] = b.astype(np.float16)
        in_maps.append(
            {
                "xT8": xT8,
                "xcm": xcm,
                "idx16": np.ascontiguousarray(idx16),
                "consts": consts,
            }
        )
    return in_maps


_CACHE = {}
TRACE = False
LAST_RESULT = None
LAST_IN_MAPS = None


def _get_nc(p, slots):
    key = (p["V"], tuple(slots))
    if key not in _CACHE:
        _CACHE[key] = build_nc(p, slots)
    return _CACHE[key]


def kernel(x, nbr_idx, deg, W, b):
    global LAST_RESULT, LAST_IN_MAPS
    x = np.asarray(x, np.float32)
    nbr_idx = np.asarray(nbr_idx, np.int32)
    deg = np.asarray(deg, np.int32)
    W = np.asarray(W, np.float32)
    b = np.asarray(b, np.float32)
    p = _plan(x.shape[2])
    orders, slots = degree_sort(deg, p)
    in_maps = host_prep(x, nbr_idx, deg, W, b, p, orders, slots)
    nc = _get_nc(p, slots)
    try:
        res = run_bass_kernel_spmd(nc, in_maps, list(range(len(in_maps))), trace=TRACE)
    except ModuleNotFoundError:
        res = run_bass_kernel_spmd(nc, in_maps, list(range(len(in_maps))), trace=False)
    LAST_RESULT = res
    LAST_IN_MAPS = in_maps
    v = p["V"]
    outs = []
    for bi, r in enumerate(res.results):
        order = orders[bi]
        valid = order < v
        ob = np.empty((O, v), np.float32)
        ob[:, order[valid]] = r["out"][:, valid].astype(np.float32)
        outs.append(ob)
    out = np.stack(outs, axis=0)
    return out[..., None].astype(np.float32)
